# revision 20
# baseline (speedup 1.0000x reference)
"""3-layer GAT (PyG-style GATConv x3 + global mean pool) on 8 trn2 NeuronCores.

Strategy: nodes are dealt round-robin by descending in-degree to the 8 cores
(dst-sharding).  ONE merged program runs all 3 layers.  Per layer, each core
runs a dense phase (hW = h @ W plus the per-node attention logit halves),
publishes fp16 "hcat" rows [hW + b | al_s] which are AllGathered across cores,
then an edge phase: for each of its nodes (128 per chunk, padded slot count K
per degree bucket) it gathers the hcat rows of the slot sources with per-slot
indirect DMAs (multi-offset batched indirect DMA is broken in the SWDGE ucode
on real HW), computes e = exp(leakyrelu(al_s + al_d)), and forms
out = (sum_k e * h + e_self * h_self) / (sum_k e + e_self) with a fold-tree
reduction; the PyG-appended self loop is handled analytically from the node's
own hcat row (an affine DMA), so it costs no gather slots.  The attention
logit projections are folded into the dense matmul (W' = [W | W@Av]); bias is
folded into hcat (softmax weights sum to 1); relu applied on DVE, and the
result transposed into SBUF as the next layer's input (double-buffered by
layer parity so dense/edge phases of adjacent layers overlap).  Layer 3
accumulates a column-sum on PE; the host divides by N and adds b3.  All
per-core programs are identical (SPMD); per-core data differs.

The kernel is bound by the SWDGE per-instruction overhead of the per-slot
indirect gathers (~1us x 1583 slots x 3 layers); to keep the Pool engine's
gather stream dense, layer l+1's dense phase is interleaved into layer l's
edge phase per chunk-group, and the per-layer AllGather is split into
decreasing-size group collectives (big groups hide under the edge phase,
the tiny last group minimizes the exposed layer-transition tail).  Each
group AllGathers into a Shared staging buffer (Shared tensors allow only
one writer instruction) and an SP-engine d2d copy lands it in the Internal
full table; layer 0 has nothing to hide behind, so it uses one big
AllGather straight into a Shared table.
"""
import numpy as np
import concourse.bass as bass
import concourse.bacc as bacc
import concourse.mybir as mybir
import concourse.tile as tile
from concourse.masks import make_identity

P = 128
NEG_SLOPE = 0.2
PAD_ALS = -30000.0  # al_s for padding rows: exp(lrelu(.)) == 0 in fp16
F32 = mybir.dt.float32
F16 = mybir.dt.float16
I32 = mybir.dt.int32
BATCH_SLOTS = 96


class Plan:
    pass


def make_plan(edge_index, N, ncores=8, kstep=1):
    E = edge_index.shape[1]
    # the PyG-appended self loop is handled analytically in the edge phase;
    # only real edges get gather slots
    src = edge_index[0].astype(np.int64)
    dst = edge_index[1].astype(np.int64)
    deg = np.bincount(dst, minlength=N)
    order = np.argsort(-(deg + 1), kind="stable")

    npc = (N + ncores - 1) // ncores
    n_chunks = (npc + P - 1) // P + 1  # last chunk is all-pad
    S = n_chunks * P

    Ks = []
    for j in range(n_chunks):
        g0 = j * P * ncores
        dmax = int(deg[order[g0]]) if g0 < N else 1
        dmax = max(dmax, 1)
        Ks.append(max(kstep, ((dmax + kstep - 1) // kstep) * kstep))
    Ks = np.array(Ks, np.int32)
    TK = int(Ks.sum())

    node_at = np.full((ncores, S), -1, np.int64)
    for c in range(ncores):
        g = np.arange(npc) * ncores + c
        valid = g < N
        node_at[c, :npc][valid] = order[g[valid]]
    row_of = np.zeros(N, np.int64)
    for c in range(ncores):
        m = node_at[c] >= 0
        row_of[node_at[c][m]] = c * S + np.nonzero(m)[0]

    eo = np.argsort(dst, kind="stable")
    src_sorted = src[eo]
    starts = np.zeros(N + 1, np.int64)
    np.cumsum(deg, out=starts[1:])

    idx = np.zeros((ncores, P, TK), np.int32)
    maskD = np.zeros((ncores, P, n_chunks), np.float32)
    off = 0
    for j in range(n_chunks):
        K = int(Ks[j])
        for c in range(ncores):
            block = np.full((P, K), c * S + (S - 1), np.int32)
            nodes = node_at[c, j * P:(j + 1) * P]
            for p in range(P):
                n = nodes[p]
                if n < 0:
                    maskD[c, p, j] = 1.0
                    continue
                s0, s1 = int(starts[n]), int(starts[n + 1])
                block[p, :s1 - s0] = row_of[src_sorted[s0:s1]]
            idx[c, :, off:off + K] = block
        off += K

    iters = []
    j = 0
    off = 0
    while j < n_chunks:
        K = int(Ks[j])
        B = max(1, BATCH_SLOTS // K)
        nb = 1
        while nb < B and j + nb < n_chunks and Ks[j + nb] == K:
            nb += 1
        iters.append((K, j, nb, off))
        off += K * nb
        j += nb

    pl = Plan()
    pl.N, pl.E, pl.ncores = N, E, ncores
    pl.npc, pl.n_chunks, pl.S, pl.TK = npc, n_chunks, S, TK
    pl.Ks, pl.iters, pl.node_at, pl.row_of = Ks, iters, node_at, row_of
    pl.idx, pl.maskD = idx, maskD
    return pl


def _fuse_W(W, a_s, a_d, heads, ch, dtype):
    """W' = [W | W@Av] where Av maps h-features to per-head src/dst logits."""
    W = np.asarray(W, np.float64)
    oc = heads * ch
    Av = np.zeros((oc, 2 * heads), np.float64)
    a_s = np.asarray(a_s, np.float64).reshape(heads, ch)
    a_d = np.asarray(a_d, np.float64).reshape(heads, ch)
    for h in range(heads):
        Av[h * ch:(h + 1) * ch, h] = a_s[h]
        Av[h * ch:(h + 1) * ch, heads + h] = a_d[h]
    return np.concatenate([W, W @ Av], axis=1).astype(dtype)


def make_inputs(pl, x, Ws, avs, bs):
    """Per-core input dict for the merged 3-layer program."""
    ins = []
    xs_all = x_slices(pl, x)
    HH, CC = (8, 8, 1), (16, 16, 32)
    Wf = [_fuse_W(Ws[l], avs[l][0], avs[l][1], HH[l], CC[l],
                  np.float32 if l == 0 else np.float16) for l in range(3)]
    for c in range(pl.ncores):
        d = {"hin": xs_all[c], "idx": pl.idx[c], "maskD": pl.maskD[c],
             "W0": Wf[0], "W1": Wf[1], "W2": Wf[2],
             "bv0": np.tile(np.asarray(bs[0], np.float32)[None, :], (P, 1)),
             "bv1": np.tile(np.asarray(bs[1], np.float32)[None, :], (P, 1))}
        ins.append(d)
    return ins


def x_slices(pl, x):
    out = []
    for c in range(pl.ncores):
        xs = np.zeros((pl.S, x.shape[1]), np.float32)
        m = pl.node_at[c] >= 0
        xs[m] = x[pl.node_at[c][m]]
        out.append(np.ascontiguousarray(xs.T))
    return out


def _dense_group(nc, ctx, l, rep, g0, g1, hT_in, alD_out):
    """Dense-phase iters for chunks [g0, g1)."""
    pl, dn, dnp = ctx["pl"], ctx["dn"], ctx["dnp"]
    OC, H, CH, RL = ctx["OC"], ctx["H"], ctx["CH"], ctx["RL"]
    oc, heads, rl = OC[l], H[l], RL[l]
    ch = CH[l]
    n_chunks, C0 = pl.n_chunks, ctx["C0"]
    hT, alD = hT_in, alD_out
    W_sb, bv_sb = ctx["W_sb"], ctx["bv_sb"]
    t_hin, hcat_loc = ctx["t_hin"], ctx["hcat_loc"]
    # ow=oc+2*heads: keep each chunk's matmul output within one 512-elem
    # PSUM bank -> DB*ow <= 512
    DB = 3
    OWMAX, RLMAX = 144, 136
    for it0 in range(g0, g1, DB):
        nb = min(DB, g1 - it0)
        xin = dn.tile([C0, DB * P], F32, tag="xin")
        if l == 0:
            nc.sync.dma_start(xin[:, :nb * P],
                              t_hin.ap()[:, it0 * P:(it0 + nb) * P])
        ow = oc + 2 * heads
        ps_t = dnp.tile([P, DB * OWMAX], F32, tag="ps")
        ps = ps_t[:, :DB * ow] if ow == OWMAX else ps_t
        for q in range(nb):
            lhsT = (xin[:, q * P:(q + 1) * P] if l == 0 else
                    hT[:, (it0 + q) * P:(it0 + q + 1) * P])
            nc.tensor.matmul(ps_t[:, q * ow:(q + 1) * ow], lhsT=lhsT,
                             rhs=W_sb[l][:], start=True, stop=True)
        psv = ps_t[:, :nb * ow].rearrange("p (q o) -> p q o", o=ow)
        dv = alD[:, it0 * 8:(it0 + nb) * 8] \
            .rearrange("p (q e) -> p q e", e=8)[:, :, :heads]
        nc.vector.tensor_copy(out=dv, in_=psv[:, :, oc + heads:oc + 2 * heads])
        hc_t = dn.tile([P, DB * RLMAX], F16, tag="hc")
        hc = hc_t[:, :DB * rl] if rl != RLMAX else hc_t
        hcv = hc[:, :nb * rl].rearrange("p (q r) -> p q r", r=rl)
        if l < 2:
            b_bc = bv_sb[l][:].unsqueeze(1).to_broadcast([P, nb, oc])
            nc.vector.tensor_tensor(out=hcv[:, :, 0:oc], in0=psv[:, :, 0:oc],
                                    in1=b_bc, op=mybir.AluOpType.add)
        else:
            nc.vector.tensor_copy(out=hcv[:, :, 0:oc], in_=psv[:, :, 0:oc])
        nc.vector.tensor_copy(
            out=hcv[:, :, oc:oc + heads], in_=psv[:, :, oc:oc + heads])
        if rl > oc + heads:
            nc.vector.memset(hcv[:, :, oc + heads:rl], 0.0)
        if it0 + nb == n_chunks:
            # last chunk is all pad rows: poison its al_s
            nc.vector.memset(
                hc[:, (nb - 1) * rl + oc:(nb - 1) * rl + oc + heads],
                PAD_ALS)
        nc.sync.dma_start(
            hcat_loc[l][:][it0 * P:(it0 + nb) * P, :]
            .rearrange("(q p) r -> p q r", p=P), hcv)


def _edge_phase(nc, ctx, l, rep):
    pl, eg, eg1, egp = ctx["pl"], ctx["eg"], ctx["eg1"], ctx["egp"]
    OC, H, CH, RL, MAXB = ctx["OC"], ctx["H"], ctx["CH"], ctx["RL"], ctx["MAXB"]
    oc, heads, ch, rl = OC[l], H[l], CH[l], RL[l]
    hT, alD, mask_sb = ctx["hT"], ctx["alD"], ctx["mask_sb"]
    hcat_loc = ctx["hcat_loc"]
    ident, ones_col = ctx["ident"], ctx["ones_col"]
    t_idx, hcat_full = ctx["t_idx"], ctx["hcat_full"]
    relu = l < 2
    if l == 2:
        ysum_ps = egp.tile([1, OC[2]], F32, tag="ysum")
        ctx["ysum_ps"] = ysum_ps
        n_mm = sum(nb for (_, _, nb, _) in pl.iters)
        i_mm = 0
    idx_all = ctx["idx_all"]
    edge_post_batch = ctx.get("edge_post_batch")
    for (K, c0, nb, coff) in pl.iters:
        ns = K * nb
        g = eg.tile([P, BATCH_SLOTS * rl], F16, tag="g")
        nq = ctx["nqueues"]
        for k in range(ns):
            _idma_q(nc.gpsimd,
                    out=g[:, k * rl:(k + 1) * rl], out_offset=None,
                    in_=hcat_full[l][:],
                    in_offset=bass.IndirectOffsetOnAxis(
                        ap=idx_all[:, coff + k:coff + k + 1], axis=0),
                    queue_num=(k % nq) if nq > 1 else 0)
        gv = g[:, :ns * rl].rearrange("p (s r) -> p s r", r=rl)
        # logits = al_s + al_d ; lrelu ; exp
        lg = eg1.tile([P, BATCH_SLOTS * 8], F16, tag="lg")
        lgv = lg[:, :ns * heads]
        al_d_bc = alD[:, c0 * 8:(c0 + nb) * 8] \
            .rearrange("p (q e) -> p q e", e=8)[:, :, :heads] \
            .unsqueeze(2).to_broadcast([P, nb, K, heads])
        nc.vector.tensor_tensor(
            out=lgv.rearrange("p (q k h) -> p q k h", k=K, h=heads),
            in0=gv[:, :, oc:oc + heads]
            .rearrange("p (q k) h -> p q k h", k=K),
            in1=al_d_bc, op=mybir.AluOpType.add)
        lg2 = eg1.tile([P, BATCH_SLOTS * 8], F16, tag="lg2")
        nc.vector.tensor_scalar_mul(lg2[:, :ns * heads], lgv, NEG_SLOPE)
        lg3 = eg1.tile([P, BATCH_SLOTS * 8], F16, tag="lg3")
        nc.vector.tensor_tensor(out=lg3[:, :ns * heads], in0=lgv,
                                in1=lg2[:, :ns * heads],
                                op=mybir.AluOpType.max)
        # e16 = exp(logits) expanded over ch (one ACT op)
        e16 = eg1.tile([P, BATCH_SLOTS * 128], F16, tag="e16")
        e16v = e16[:, :ns * oc]
        nc.scalar.activation(
            out=e16v.rearrange("p (s h c) -> p s h c", h=heads, c=ch),
            in_=lg3[:, :ns * heads].rearrange("p (s h) -> p s h", h=heads)
            .unsqueeze(3).to_broadcast([P, ns, heads, ch]),
            func=mybir.ActivationFunctionType.Exp)
        den = eg1.tile([P, MAXB * 8], F32, tag="den")
        nc.vector.tensor_reduce(
            out=den[:, :nb * heads],
            in_=e16[:, :ns * oc].rearrange(
                "p (q k h c) -> p q h k c", k=K, h=heads, c=ch)[:, :, :, :, 0],
            axis=mybir.AxisListType.X, op=mybir.AluOpType.add)
        # analytic self-loop term: e_self = exp(lrelu(al_s + al_d)); the
        # node's own hcat row (h+b | al_s) is an affine DMA from hcat_loc
        selfr = eg.tile([P, MAXB * 136], F16, tag="selfr")
        sv = selfr[:, :nb * rl].rearrange("p (q r) -> p q r", r=rl)
        nc.sync.dma_start(
            sv, hcat_loc[l][:][c0 * P:(c0 + nb) * P, :]
            .rearrange("(q p) r -> p q r", p=P))
        sl = eg1.tile([P, MAXB * 8], F16, tag="sl")
        nc.vector.tensor_tensor(
            out=sl[:, :nb * heads].rearrange("p (q h) -> p q h", h=heads),
            in0=sv[:, :, oc:oc + heads],
            in1=alD[:, c0 * 8:(c0 + nb) * 8]
            .rearrange("p (q e) -> p q e", e=8)[:, :, :heads],
            op=mybir.AluOpType.add)
        sl2 = eg1.tile([P, MAXB * 8], F16, tag="sl2")
        nc.vector.tensor_scalar_mul(sl2[:, :nb * heads], sl[:, :nb * heads],
                                    NEG_SLOPE)
        sl3 = eg1.tile([P, MAXB * 8], F16, tag="sl3")
        nc.vector.tensor_tensor(out=sl3[:, :nb * heads], in0=sl[:, :nb * heads],
                                in1=sl2[:, :nb * heads], op=mybir.AluOpType.max)
        sl4 = eg1.tile([P, MAXB * 8], F16, tag="sl4")
        nc.vector.tensor_tensor(
            out=sl4[:, :nb * heads].rearrange("p (q h) -> p q h", h=heads),
            in0=sl3[:, :nb * heads].rearrange("p (q h) -> p q h", h=heads),
            in1=ctx["maskP"][:, c0:c0 + nb].unsqueeze(2)
            .to_broadcast([P, nb, heads]),
            op=mybir.AluOpType.add)
        eself = eg1.tile([P, MAXB * 128], F16, tag="eself")
        nc.scalar.activation(
            out=eself[:, :nb * oc].rearrange(
                "p (q h c) -> p q h c", h=heads, c=ch),
            in_=sl4[:, :nb * heads].rearrange("p (q h) -> p q h", h=heads)
            .unsqueeze(3).to_broadcast([P, nb, heads, ch]),
            func=mybir.ActivationFunctionType.Exp)
        den2 = eg1.tile([P, MAXB * 8], F32, tag="den2")
        m_bc = mask_sb[:, c0:c0 + nb].unsqueeze(2) \
            .to_broadcast([P, nb, heads])
        nc.vector.tensor_tensor(
            out=den2[:, :nb * heads].rearrange("p (q h) -> p q h", h=heads),
            in0=den[:, :nb * heads].rearrange("p (q h) -> p q h", h=heads),
            in1=m_bc, op=mybir.AluOpType.add)
        den3 = eg1.tile([P, MAXB * 8], F32, tag="den3")
        nc.vector.tensor_tensor(
            out=den3[:, :nb * heads].rearrange("p (q h) -> p q h", h=heads),
            in0=den2[:, :nb * heads].rearrange("p (q h) -> p q h", h=heads),
            in1=eself[:, :nb * oc].rearrange(
                "p (q h c) -> p q h c", h=heads, c=ch)[:, :, :, 0],
            op=mybir.AluOpType.add)
        inv = eg1.tile([P, MAXB * 8], F32, tag="inv")
        nc.vector.reciprocal(out=inv[:, :nb * heads], in_=den3[:, :nb * heads])
        mp = eg1.tile([P, BATCH_SLOTS * 128], F16, tag="mp")
        nc.vector.tensor_tensor(
            out=mp[:, :ns * oc].rearrange("p (s c) -> p s c", c=oc),
            in0=gv[:, :, 0:oc],
            in1=e16v.rearrange("p (s c) -> p s c", c=oc),
            op=mybir.AluOpType.mult)
        # fold-tree reduce over K -> msum
        scrA = eg1.tile([P, (BATCH_SLOTS // 2 + 8) * 128], F16, tag="scrA")
        scrB = mp  # level k reads only level k-1; mp is dead after level 1
        cur, curk = mp, K
        while curk > 1:
            a_in = cur[:, :nb * curk * oc].rearrange(
                "p (q k c) -> p q k c", k=curk, c=oc)
            if curk % 2 == 1:
                half = (curk + 1) // 2
                pair = curk - half
            else:
                half, pair = curk // 2, curk // 2
            dst_t = scrA if cur is not scrA else scrB
            o_v = dst_t[:, :nb * half * oc].rearrange(
                "p (q k c) -> p q k c", k=half, c=oc)
            nc.vector.tensor_tensor(
                out=o_v[:, :, 0:pair], in0=a_in[:, :, 0:pair],
                in1=a_in[:, :, half:half + pair], op=mybir.AluOpType.add)
            if half > pair:
                nc.vector.tensor_copy(out=o_v[:, :, pair:half],
                                      in_=a_in[:, :, pair:half])
            cur, curk = dst_t, half
        mp_s = eg1.tile([P, MAXB * 128], F16, tag="mps")
        nc.vector.tensor_tensor(
            out=mp_s[:, :nb * oc].rearrange("p (q c) -> p q c", c=oc),
            in0=eself[:, :nb * oc].rearrange("p (q c) -> p q c", c=oc),
            in1=sv[:, :, 0:oc],
            op=mybir.AluOpType.mult)
        cur2 = eg1.tile([P, MAXB * 128], F16, tag="cur2")
        nc.vector.tensor_tensor(
            out=cur2[:, :nb * oc], in0=cur[:, :nb * oc],
            in1=mp_s[:, :nb * oc], op=mybir.AluOpType.add)
        cur = cur2
        # normalize (+relu)
        hout = eg.tile([P, MAXB * 128], F16, tag="hout")
        inv_bc = inv[:, :nb * heads].rearrange(
            "p (q h) -> p q h", h=heads).unsqueeze(3).to_broadcast(
            [P, nb, heads, ch])
        nc.vector.tensor_tensor(
            out=hout[:, :nb * oc].rearrange(
                "p (q h c) -> p q h c", h=heads, c=ch),
            in0=cur[:, :nb * oc].rearrange(
                "p (q h c) -> p q h c", h=heads, c=ch),
            in1=inv_bc, op=mybir.AluOpType.mult)
        if relu:
            hr = eg.tile([P, MAXB * 128], F16, tag="hr")
            nc.vector.tensor_scalar_max(hr[:, :nb * oc], hout[:, :nb * oc], 0.0)
            hT_out = ctx["hT_out"]
            for q in range(nb):
                tp = egp.tile([P, P], F16, tag="tp")
                nc.tensor.transpose(out=tp[:], in_=hr[:, q * oc:(q + 1) * oc],
                                    identity=ident[:])
                nc.vector.tensor_copy(
                    out=hT_out[:, (c0 + q) * P:(c0 + q + 1) * P], in_=tp[:])
        else:
            for q in range(nb):
                nc.tensor.matmul(
                    ysum_ps[:], lhsT=ones_col[:],
                    rhs=hout[:, q * oc:(q + 1) * oc],
                    start=(i_mm == 0), stop=(i_mm == n_mm - 1))
                i_mm += 1
        if edge_post_batch is not None:
            edge_post_batch(c0 + nb)




def _idma_q(gp, out, out_offset, in_, in_offset, queue_num=0):
    """indirect_dma_start with a selectable SWDGE queue (qPoolDynamic{n})."""
    self = gp
    from concourse.bass import MemorySpace
    assert (out_offset is not None) ^ (in_offset is not None)
    if in_offset is not None:
        assert in_.space == MemorySpace.DRAM and out.space == MemorySpace.SBUF
        src_ap, dest_ap = in_, out
    else:
        assert out.space == MemorySpace.DRAM and in_.space == MemorySpace.SBUF
        src_ap, dest_ap = out, in_
    offset_ap_with_axis = in_offset or out_offset
    offset_ap = offset_ap_with_axis.ap
    offset_axis = offset_ap_with_axis.axis
    assert isinstance(src_ap.offset, int) and src_ap.offset == 0
    out_ap = self.lower_ap_dma(out, for_indirect_dma=True)
    in_ap = self.lower_ap_dma(in_, for_indirect_dma=True)
    assert len(in_ap) == 1 and len(out_ap) == 1
    offset_ap = self.lower_ap_dma(offset_ap)
    assert len(offset_ap) == 1
    in_ap.append(offset_ap[0])
    ap_shape = src_ap.shape
    coef = 1
    for i in range(offset_axis + 1, len(ap_shape)):
        coef *= ap_shape[i]
    dynamic_ap_info = mybir.DynamicAccessPatternInfo(
        c=0, actual_ap=dest_ap.ap,
        indirect_dim_max_index=ap_shape[offset_axis],
        offset_expr=[mybir.DynamicAccessPatternOffsetExpr(
            coef=coef,
            aff_expr=mybir.DynamicAccessPatternOffsetExprAffExpr(
                kind="IndirectArgId", arg_id=1))])
    if in_offset:
        in_ap[0].dynamic_ap_info = dynamic_ap_info
    else:
        out_ap[0].dynamic_ap_info = dynamic_ap_info
    return self.add_instruction(
        mybir.InstDMACopy(
            name=self.bass.get_next_instruction_name(),
            queue=f"qPoolDynamic{queue_num or ''}",
            mode="Copy", ins=in_ap, outs=out_ap,
            oob_is_err=True, cce_op=mybir.AluOpType.bypass))

def build_program(pl, C0=128, H=(8, 8, 1), CH=(16, 16, 32), ncores=8, repeat=1, nqueues=4):
    OC = [H[i] * CH[i] for i in range(3)]
    RL = [((OC[i] + H[i] + 1) // 2) * 2 for i in range(3)]  # 136,136,34
    S, n_chunks = pl.S, pl.n_chunks
    NC = ncores
    MAXB = max(nb for (_, _, nb, _) in pl.iters)

    nc = bacc.Bacc("TRN2", target_bir_lowering=False, debug=False, num_devices=NC,
                   num_swdge_queues=max(nqueues, 1))
    t_hin = nc.dram_tensor("hin", [C0, S], F32, kind="ExternalInput")
    t_idx = nc.dram_tensor("idx", [P, pl.TK], I32, kind="ExternalInput")
    t_maskD = nc.dram_tensor("maskD", [P, n_chunks], F32, kind="ExternalInput")
    t_W = [nc.dram_tensor("W0", [C0, OC[0] + 2 * H[0]], F32, kind="ExternalInput"),
           nc.dram_tensor("W1", [OC[0], OC[1] + 2 * H[1]], F16, kind="ExternalInput"),
           nc.dram_tensor("W2", [OC[1], OC[2] + 2 * H[2]], F16, kind="ExternalInput")]
    t_bv = [nc.dram_tensor(f"bv{l}", [P, OC[l]], F32, kind="ExternalInput")
            for l in range(2)]
    t_y = nc.dram_tensor("y", [1, OC[2]], F32, kind="ExternalOutput")

    with tile.TileContext(nc) as tc:
        with tc.tile_pool(name="res", bufs=1) as res, \
             tc.tile_pool(name="dram", bufs=1, space="DRAM") as dram:
            hTb = [res.tile([P, S], F16, name="hT0"),
                   res.tile([P, S], F16, name="hT1")]
            alDb = [res.tile([P, n_chunks * 8], F16, name="alD0"),
                    res.tile([P, n_chunks * 8], F16, name="alD1")]
            idx_all = res.tile([P, pl.TK], I32)
            nc.sync.dma_start(idx_all[:], t_idx.ap())
            mask_sb = res.tile([P, n_chunks], F32)
            nc.sync.dma_start(mask_sb[:], t_maskD.ap())
            maskP = res.tile([P, n_chunks], F16)
            nc.vector.tensor_scalar_mul(maskP[:], mask_sb[:], PAD_ALS)
            ident = res.tile([P, P], F16)
            make_identity(nc, ident[:])
            ones_col = res.tile([P, 1], F16)
            nc.gpsimd.memset(ones_col[:], 1.0)
            W_sb, bv_sb = {}, {}
            for l in range(3):
                W_sb[l] = res.tile(list(t_W[l].shape), F32 if l == 0 else F16,
                                   name=f"W{l}sb")
                nc.sync.dma_start(W_sb[l][:], t_W[l].ap())
                if l < 2:
                    bv_sb[l] = res.tile([P, OC[l]], F32, name=f"bv{l}sb")
                    nc.sync.dma_start(bv_sb[l][:], t_bv[l].ap())

            hcat_loc = {l: dram.tile([S, RL[l]], F16, name=f"hcl{l}")
                        for l in range(3)}
            # Chunked AllGather for steady-state layers: groups fire as the
            # interleaved dense iters finish, hiding the collective chain
            # under the previous layer's gather stream.  Sizes DECREASE so
            # only a tiny last group is exposed at the layer transition.
            GB = [0, 24, 48, 66, 81, 90, 96, n_chunks]
            # layer 0 (first rep) uses one big AllGather straight into its
            # Shared full table -- nothing to hide behind at program start,
            # and one large collective beats a chain of small ones.
            hcat_full_r = {}
            for rep in range(repeat):
                for l in range(3):
                    kw = {"addr_space": "Shared"} if (rep, l) == (0, 0) else {}
                    hcat_full_r[(rep, l)] = dram.tile(
                        [NC * S, RL[l]], F16, name=f"hcf{rep}_{l}", **kw)
            # a Shared tensor may be written by exactly one instruction, so
            # each (rep, layer, group) AllGather gets its own staging buffer
            ag_stage = {(rep, l, g): dram.tile(
                            [NC * (GB[g + 1] - GB[g]) * P, RL[l]], F16,
                            name=f"st{rep}_{l}_{g}", addr_space="Shared")
                        for rep in range(repeat) for l in range(3)
                        for g in range(len(GB) - 1)
                        if (rep, l) != (0, 0)}

            ctx = dict(pl=pl, OC=OC, H=H, CH=CH, RL=RL, MAXB=MAXB, C0=C0,
                       nqueues=nqueues, idx_all=idx_all,
                       mask_sb=mask_sb, maskP=maskP, ident=ident,
                       ones_col=ones_col, W_sb=W_sb, bv_sb=bv_sb, t_hin=t_hin,
                       t_idx=t_idx, hcat_loc=hcat_loc)

            NG = len(GB) - 1
            GL = 3 * repeat

            with tc.tile_pool(name="dn", bufs=2) as dn, \
                 tc.tile_pool(name="dnp", bufs=2, space="PSUM") as dnp:
              ctx["dn"], ctx["dnp"] = dn, dnp

              def emit_dense_group(gl, g):
                  """Dense iters + AllGather + table copy for chunk group g of
                  global layer gl.  Emitted inline into the previous layer's
                  edge phase so the collectives overlap the gather stream."""
                  rep1, l1 = divmod(gl, 3)
                  _dense_group(nc, ctx, l1, rep1, GB[g], GB[g + 1],
                               hT_in=hTb[gl % 2], alD_out=alDb[gl % 2])
                  r0, r1 = GB[g] * P, GB[g + 1] * P
                  stage = ag_stage[(rep1, l1, g)]
                  nc.gpsimd.collective_compute(
                      "AllGather", mybir.AluOpType.bypass,
                      replica_groups=[list(range(NC))],
                      ins=[hcat_loc[l1][:][r0:r1, :]],
                      outs=[stage[:]])
                  full = hcat_full_r[(rep1, l1)]
                  nc.sync.dma_start(
                      full[:].rearrange("(c s) r -> c s r", c=NC)[:, r0:r1, :],
                      stage[:].rearrange("(c s) r -> c s r", c=NC))

              # prologue: full layer-0 dense, then one big AllGather
              _dense_group(nc, ctx, 0, 0, 0, n_chunks,
                           hT_in=hTb[0], alD_out=alDb[0])
              nc.gpsimd.collective_compute(
                  "AllGather", mybir.AluOpType.bypass,
                  replica_groups=[list(range(NC))],
                  ins=[hcat_loc[0][:]],
                  outs=[hcat_full_r[(0, 0)][:]])
              for gl in range(GL):
                rep, l = divmod(gl, 3)
                ctx["hcat_full"] = {l: hcat_full_r[(rep, l)]}
                ctx["hT"] = hTb[gl % 2]
                ctx["alD"] = alDb[gl % 2]
                ctx["hT_out"] = hTb[(gl + 1) % 2]  # edge output
                state = {"g": 0}

                def edge_post_batch(end_chunk, gl=gl, state=state):
                    while (state["g"] < NG
                           and end_chunk >= GB[state["g"] + 1]):
                        if gl + 1 < GL:
                            emit_dense_group(gl + 1, state["g"])
                        state["g"] += 1

                ctx["edge_post_batch"] = edge_post_batch
                with tc.tile_pool(name=f"eg_{rep}_{l}", bufs=2) as eg, \
                     tc.tile_pool(name=f"eg1_{rep}_{l}", bufs=1) as eg1, \
                     tc.tile_pool(name=f"egp_{rep}_{l}", bufs=2,
                                  space="PSUM") as egp:
                    ctx["eg"], ctx["eg1"], ctx["egp"] = eg, eg1, egp
                    _edge_phase(nc, ctx, l, rep)
                    if l == 2:
                        ysb = res.tile([1, OC[2]], F32, name=f"ysb{rep}")
                        nc.vector.tensor_copy(out=ysb[:],
                                              in_=ctx["ysum_ps"][:])
                        nc.sync.dma_start(t_y.ap(), ysb[:])
    nc.compile()
    return nc


# ----------------------------------------------------------------- entry point

N_NODES, N_EDGES = 100000, 1600000
_CACHE = {}


def _get_compiled(edge_index):
    key = hash(edge_index.tobytes())
    if key not in _CACHE:
        pl = make_plan(edge_index, N_NODES, ncores=8)
        prog = build_program(pl, C0=128, H=(8, 8, 1), CH=(16, 16, 32), ncores=8)
        _CACHE.clear()
        _CACHE[key] = (pl, prog)
    return _CACHE[key]


def kernel(x, edge_index, W1, a_src1, a_dst1, b1, W2, a_src2, a_dst2, b2,
           W3, a_src3, a_dst3, b3):
    from concourse import bass_utils
    x = np.asarray(x, np.float32)
    edge_index = np.asarray(edge_index, np.int32)
    pl, prog = _get_compiled(edge_index)
    in_maps = make_inputs(pl, x, (W1, W2, W3),
                          ((a_src1, a_dst1), (a_src2, a_dst2), (a_src3, a_dst3)),
                          (b1, b2, None))
    res = bass_utils.run_bass_kernel_spmd(prog, in_maps, core_ids=list(range(8)))
    tot = np.sum([res.results[c]["y"] for c in range(8)], axis=0)
    return (tot / np.float32(N_NODES)
            + np.asarray(b3, np.float32)[None, :]).astype(np.float32)

